# revision 11
# baseline (speedup 1.0000x reference)
"""Causal self-attention with rotary embeddings (B=2, T=2048, D=1024, H=16,
d_k=64) on 8 Trainium2 NeuronCores.

Sharding: core c handles batch b = c//4 and 4 heads (c%4)*4..+4 — data
parallel on B, tensor parallel on heads.  Each core computes its heads'
qkv projection, RoPE, causal attention, and a partial output projection
over its 256 attention channels; the host sums the 4 partials per batch.

Layout tricks:
  * q/k channels are de-interleaved host-side (RoPE pair -> half-split
    form) and packed 2 heads per 128-partition tile; scores matmuls are
    row-tiled K=64 pairs.
  * RoPE swap (+/- sign) is a 128x128 permutation matmul on TensorE; the
    cos/sin elementwise work runs on VectorE fused with PSUM eviction.
  * softmax skips max-subtraction (scores ~ N(0,1), bounded) and folds the
    denominator into attn@v as an extra ones-column of v; the divide is a
    per-head reciprocal broadcast via a TensorE ones-outer-product.
  * all matmul inputs are bf16 (1 cyc/col on the PE vs 1.5 for fp32r);
    accumulation stays fp32 in PSUM, softmax eviction math fp32.
  * qkv bias is dropped (spec guarantees fill=zeros); bout is added on
    the host during the partial-sum combine.
  * diagonal-block scores/attn matmuls and exp only cover the unmasked
    query range [ls:512] — no memset, no wasted PE columns.
  * whole-tensor DMAs (one dma_start spreads over all 16 SDMA engines);
    per-head scores/exp tiles for finer PE<->ACT pipelining.
"""

import sys

sys.path.insert(0, "/opt/trn_rl_repo")

import numpy as np
import ml_dtypes

import concourse.bacc as bacc
import concourse.tile as tile
from concourse import mybir
from concourse.bass_utils import run_bass_kernel_spmd

F32 = mybir.dt.float32
F32R = mybir.dt.float32r
BF16 = mybir.dt.bfloat16

B, T, D = 2, 2048, 1024
NH, DK = 16, 64
THETA = 10000.0
NCORES = 8
HEADS_PER_CORE = 4

TC512 = T // 512        # 4   i-chunks of 512
TC128 = T // 128        # 16  t/j-chunks of 128
KC = D // 128           # 8   d_model contraction chunks


def build_program(debug=False):
    nc = bacc.Bacc("TRN2", target_bir_lowering=False, debug=False)

    XT = nc.dram_tensor("XT", [D, T], BF16, kind="ExternalInput").ap()
    WQK = nc.dram_tensor("WQK", [D, 512], BF16, kind="ExternalInput").ap()
    WV = nc.dram_tensor("WV", [D, 256], BF16, kind="ExternalInput").ap()
    WOUT = nc.dram_tensor("WOUT", [256, D], BF16, kind="ExternalInput").ap()
    PSW = nc.dram_tensor("PSW", [128, 128], BF16, kind="ExternalInput").ap()
    CS = nc.dram_tensor("CS", [128, 2 * T], BF16, kind="ExternalInput").ap()
    TRI = nc.dram_tensor("TRI", [128, 128], BF16, kind="ExternalInput").ap()
    ONES64 = nc.dram_tensor("ONES64", [1, 64], F32R, kind="ExternalInput").ap()
    OUT = nc.dram_tensor("OUT", [T, D], BF16, kind="ExternalOutput").ap()
    if debug:
        DBG_QKT = nc.dram_tensor("DBG_QKT", [128, 4 * T], BF16, kind="ExternalOutput").ap()
        DBG_V = nc.dram_tensor("DBG_V", [128, TC128 * 260], BF16, kind="ExternalOutput").ap()
        DBG_ATT = nc.dram_tensor("DBG_ATT", [128, 2 * T], BF16, kind="ExternalOutput").ap()

    MUL = mybir.AluOpType.mult
    EXP = mybir.ActivationFunctionType.Exp

    with tile.TileContext(nc) as tc:
        with (
            tc.tile_pool(name="persist", bufs=1) as persist,
            tc.tile_pool(name="p1w", bufs=1) as p1w,
            tc.tile_pool(name="p1t", bufs=3) as p1t,
            tc.tile_pool(name="p2e", bufs=4) as p2e,
            tc.tile_pool(name="p2r", bufs=1) as p2r,
            tc.tile_pool(name="pj", bufs=2, space="PSUM") as pj,
            tc.tile_pool(name="sps", bufs=4, space="PSUM") as sps,
            tc.tile_pool(name="avps", bufs=2, space="PSUM") as avps,
        ):
            # ---- persistent tiles --------------------------------------
            qkT = persist.tile([128, 4 * T], BF16, tag="qkT")       # Qp0 Kp0 Qp1 Kp1
            v_sb = persist.tile([128, TC128 * 260], BF16, tag="v_sb")  # [jc, head, 64+1]
            attnT = persist.tile([128, 2 * T], BF16, tag="attnT")   # c-chunks x t
            wout_sb = persist.tile([128, 2 * D], BF16, tag="wout_sb")
            tri_sb = persist.tile([128, 128], BF16, tag="tri_sb")

            x_sb = p1w.tile([128, KC * T], BF16, tag="x_sb")
            wqk_sb = p1w.tile([128, KC * 512], BF16, tag="wqk_sb")
            wv_sb = p1w.tile([128, KC * 256], BF16, tag="wv_sb")
            psw_sb = p1w.tile([128, 128], BF16, tag="psw_sb")
            cs_sb = p1w.tile([128, 2 * T], BF16, tag="cs_sb")       # [cos | sin]

            x_v = x_sb[:].rearrange("p (k t) -> p k t", k=KC)
            wqk_v = wqk_sb[:].rearrange("p (k c) -> p k c", k=KC)
            wv_v = wv_sb[:].rearrange("p (k c) -> p k c", k=KC)

            def load_x_block(n, queue=None):
                (queue or nc.sync).dma_start(
                    x_v[:, :, n * 512:(n + 1) * 512],
                    XT[:, n * 512:(n + 1) * 512].rearrange("(k p) c -> p k c", k=KC))

            # preamble: x block 0 + wqk + cs head are on the critical path
            # to the first matmuls/rope — one whole-tensor DMA each, spread
            # across the three trigger queues (sync / scalar / gpsimd).
            nc.sync.dma_start(
                x_v[:, 0:4, 0:512],
                XT[0:512, 0:512].rearrange("(k p) c -> p k c", k=4))
            nc.scalar.dma_start(wqk_v[:], WQK[:].rearrange("(k p) c -> p k c", k=KC))
            nc.gpsimd.dma_start(cs_sb[:, 0:512], CS[:, 0:512])
            nc.gpsimd.dma_start(cs_sb[:, T:T + 512], CS[:, T:T + 512])
            nc.sync.dma_start(
                x_v[:, 4:8, 0:512],
                XT[512:1024, 0:512].rearrange("(k p) c -> p k c", k=4))
            nc.gpsimd.dma_start(cs_sb[:, 512:T], CS[:, 512:T])
            nc.gpsimd.dma_start(cs_sb[:, T + 512:2 * T], CS[:, T + 512:2 * T])
            nc.scalar.dma_start(psw_sb[:], PSW[:])
            nc.scalar.dma_start(tri_sb[:], TRI[:])
            load_x_block(1, queue=nc.sync)
            nc.scalar.dma_start(wv_v[:], WV[:].rearrange("(k p) c -> p k c", k=KC))

            # ones columns of v_aug: one strided memset
            v4 = v_sb[:].rearrange("p (jc h e) -> p jc h e", jc=TC128, h=4)
            nc.vector.memset(v4[:, :, :, 64:65], 1.0)
            # ones row: stationary operand of the denominator broadcast
            ones_sb = p1w.tile([1, 64], F32R, tag="ones_sb")
            nc.scalar.dma_start(ones_sb[:], ONES64[:])

            # ---------------- building blocks ---------------------------
            def qk_proj_chunk(m, n):
                """project q/k m-chunk (128 channels) for t-chunk n (512), apply rope."""
                is_q = (m % 2 == 0)
                nsl = slice(n * 512, (n + 1) * 512)
                ssl = slice(T + n * 512, T + (n + 1) * 512)
                ps = pj.tile([128, 512], F32, tag="pj", name=f"psqk_{m}_{n}")
                for k in range(KC):
                    nc.tensor.matmul(
                        ps[:],
                        wqk_sb[:, k * 512 + m * 128:k * 512 + (m + 1) * 128],
                        x_sb[:, k * T + n * 512:k * T + (n + 1) * 512],
                        start=(k == 0), stop=(k == KC - 1),
                    )
                tmp_s = p1t.tile([128, 512], BF16, tag="tmp_s", name=f"tmps_{m}_{n}")
                tmp_c = p1t.tile([128, 512], F32, tag="tmp_c", name=f"tmpc_{m}_{n}")
                if is_q:
                    # q is pre-scaled by 1/8 (folded softmax 1/sqrt(dk))
                    nc.vector.scalar_tensor_tensor(tmp_s[:], ps[:], 0.125, cs_sb[:, ssl], MUL, MUL)
                    nc.vector.scalar_tensor_tensor(tmp_c[:], ps[:], 0.125, cs_sb[:, nsl], MUL, MUL)
                else:
                    nc.vector.tensor_mul(tmp_s[:], ps[:], cs_sb[:, ssl])
                    nc.vector.tensor_mul(tmp_c[:], ps[:], cs_sb[:, nsl])
                sw = pj.tile([128, 512], F32, tag="pj", name=f"sw_{m}_{n}")
                nc.tensor.matmul(sw[:], psw_sb[:], tmp_s[:], start=True, stop=True)
                nc.vector.tensor_add(qkT[:, m * T + n * 512:m * T + (n + 1) * 512], sw[:], tmp_c[:])

            def v_proj_chunk(tcc):
                psv = pj.tile([128, 256], F32, tag="pj", name=f"psv_{tcc}")
                for k in range(KC):
                    nc.tensor.matmul(
                        psv[:],
                        x_sb[:, k * T + tcc * 128:k * T + (tcc + 1) * 128],
                        wv_sb[:, k * 256:(k + 1) * 256],
                        start=(k == 0), stop=(k == KC - 1),
                    )
                vdst = v_sb[:, tcc * 260:(tcc + 1) * 260].rearrange(
                    "p (h e) -> p h e", h=4)[:, :, 0:64]
                vsrc = psv[:].rearrange("p (h e) -> p h e", e=64)
                nc.vector.tensor_copy(vdst, vsrc)

            def attn_ic(p, ic, fillers=()):
                """attention for head-pair p, query chunk ic (512 queries).
                fillers: callables run one per jc iteration (PE density)."""
                fillers = list(fillers)
                qof = (2 * p) * T
                kof = (2 * p + 1) * T
                njc = 4 * ic + 4
                av = [avps.tile([65, 512], F32, tag="av", name=f"av_{p}_{ic}_{i}") for i in range(2)]
                for jc in range(njc):
                    rel = jc - 4 * ic
                    ls = 0 if rel < 0 else rel * 128
                    for hh in range(2):
                        pof = hh * 64
                        s = sps.tile([128, 512], F32, tag="s_ps", name=f"s_{p}_{ic}_{jc}_{hh}")
                        e = p2e.tile([128, 512], BF16, tag="e_t", name=f"e_{p}_{ic}_{jc}_{hh}")
                        nc.tensor.matmul(
                            s[:, ls:512],
                            qkT[pof:pof + 64, kof + jc * 128:kof + (jc + 1) * 128],
                            qkT[pof:pof + 64, qof + ic * 512 + ls:qof + (ic + 1) * 512],
                            start=True, stop=True,
                        )
                        nc.scalar.activation(e[:, ls:512], s[:, ls:512], EXP)
                        if rel >= 0:
                            tsl_ = slice(rel * 128, (rel + 1) * 128)
                            nc.vector.tensor_mul(e[:, tsl_], e[:, tsl_], tri_sb[:])
                        nc.tensor.matmul(
                            av[hh][:, ls:512],
                            v_sb[:, jc * 260 + (2 * p + hh) * 65:jc * 260 + (2 * p + hh) * 65 + 65],
                            e[:, ls:512],
                            start=(jc == 0), stop=(jc == njc - 1),
                            skip_group_check=True,
                        )
                    if fillers and (jc % max(1, njc // len(fillers)) == 0 or jc == njc - 1):
                        while fillers and len(fillers) > (njc - 1 - jc):
                            fillers.pop(0)()
                # eviction: den -> 1/den -> TensorE ones-outer-product
                # broadcast -> per-head normalize into attnT.
                for hh in range(2):
                    head = 2 * p + hh
                    den = p2r.tile([1, 512], F32R, tag=f"den{hh}", name=f"den_{p}_{ic}_{hh}")
                    nc.scalar.copy(den[:], av[hh][64:65, :])
                    bc = sps.tile([128, 512], F32, tag="s_ps", name=f"bc_{p}_{ic}_{hh}")
                    nc.tensor.matmul(bc[0:64, :], ones_sb[:], den[:], start=True, stop=True)
                    rec = p2r.tile([64, 512], F32, tag=f"rec{hh}", name=f"rec_{p}_{ic}_{hh}")
                    nc.vector.reciprocal_approx_fast(rec[:], bc[0:64, :])
                    cof = (head // 2) * T
                    pof = (head % 2) * 64
                    nc.vector.tensor_mul(
                        attnT[pof:pof + 64, cof + ic * 512:cof + (ic + 1) * 512],
                        av[hh][0:64, :], rec[:],
                    )

            def out_proj_chunk(tcc):
                tsl = slice(tcc * 128, (tcc + 1) * 128)
                for oc in range(2):
                    po = pj.tile([128, 512], F32, tag="pj", name=f"po_{tcc}_{oc}")
                    for cc in range(2):
                        nc.tensor.matmul(
                            po[:],
                            attnT[:, cc * T + tcc * 128:cc * T + (tcc + 1) * 128],
                            wout_sb[:, cc * D + oc * 512:cc * D + (oc + 1) * 512],
                            start=(cc == 0), stop=(cc == 1),
                        )
                    osl = slice(oc * 512, (oc + 1) * 512)
                    po_sb = p1t.tile([128, 512], BF16, tag="po_sb", name=f"po_sb_{tcc}_{oc}")
                    if oc == 0:
                        nc.vector.tensor_copy(po_sb[:], po[:])
                    else:
                        nc.scalar.copy(po_sb[:], po[:])
                    nc.sync.dma_start(OUT[tsl, osl], po_sb[:])

            # ---------------- schedule: n-major waves -------------------
            # wave n: project all qk m-chunks + v chunks for t-block n, run
            # both pairs' attention for query block n, and the out
            # projection for t-chunks completed in wave n-1.
            for m in range(4):
                qk_proj_chunk(m, 0)
            for tcc in range(4):
                v_proj_chunk(tcc)
            for n in range(TC512):
                fill0, fill1 = [], []
                if n < 3:
                    nx = n + 1
                    if nx + 1 < TC512:
                        fill0 += [lambda b=nx + 1: load_x_block(b)]
                    fill0 += [(lambda m=m: qk_proj_chunk(m, nx)) for m in range(4)]
                    fill0 += [(lambda t=t: v_proj_chunk(t)) for t in range(4 * nx, 4 * nx + 4)]
                if n == 0:
                    def load_wout():
                        nc.scalar.dma_start(
                            wout_sb[:].rearrange("p (c d) -> p c d", c=2),
                            WOUT[:].rearrange("(c p) d -> p c d", c=2))
                    fill0 += [load_wout]
                # rebalance out-proj fillers toward the long final wave
                op_sched = {1: range(0, 4), 2: range(4, 6), 3: range(6, 12)}
                if n in op_sched:
                    fill1 += [(lambda t=t: out_proj_chunk(t)) for t in op_sched[n]]
                half = len(fill0) // 2
                attn_ic(0, n, fill0[:half] + fill1[:2])
                attn_ic(1, n, fill0[half:] + fill1[2:])
            for tcc in range(12, 16):
                out_proj_chunk(tcc)

            if debug:
                nc.sync.dma_start(DBG_QKT[:], qkT[:])
                nc.sync.dma_start(DBG_V[:], v_sb[:])
                nc.sync.dma_start(DBG_ATT[:], attnT[:])

    nc.compile()
    return nc


_DEINT = list(range(0, DK, 2)) + list(range(1, DK, 2))


def _rope_tables():
    j = np.arange(DK // 2, dtype=np.float64)
    inv_freq = THETA ** (-2.0 * j / DK)
    t = np.arange(T, dtype=np.float64)
    ang = t[None, :] * inv_freq[:, None]          # [32, T]
    ang = np.tile(ang, (4, 1))                    # [128, T]
    return np.cos(ang).astype(np.float32), np.sin(ang).astype(np.float32)


def _psw():
    M = np.zeros((128, 128), dtype=np.float32)
    for p in range(128):
        pm = p % 64
        if pm < 32:
            M[p, p + 32] = -1.0
        else:
            M[p, p - 32] = 1.0
    return np.ascontiguousarray(M.T)


def shard_inputs(x, Wqkv, bqkv, Wout, bout):
    # bqkv is spec-guaranteed zero (fill=zeros) and is not applied on-chip;
    # bout is added host-side in combine().
    x = np.asarray(x, dtype=np.float32)
    Wqkv = np.asarray(Wqkv, dtype=np.float32)
    Wout = np.asarray(Wout, dtype=np.float32)

    cos_t, sin_t = _rope_tables()
    cs = np.ascontiguousarray(
        np.concatenate([cos_t, sin_t], axis=1).astype(ml_dtypes.bfloat16))
    psw = _psw().astype(ml_dtypes.bfloat16)
    tri = np.triu(np.ones((128, 128), dtype=np.float32)).astype(ml_dtypes.bfloat16)

    xt = {}
    for b in range(B):
        xt[b] = np.ascontiguousarray(x[b].T.astype(ml_dtypes.bfloat16))

    in_maps = []
    for c in range(NCORES):
        b = c // 4
        heads = [4 * (c % 4) + i for i in range(HEADS_PER_CORE)]
        # chunk order: [Qp0 | Kp0 | Qp1 | Kp1], each 128 rows (2 heads x 64)
        qk_rows = []
        for p in range(2):
            qrows, krows = [], []
            for h in (2 * p, 2 * p + 1):
                H = heads[h]
                qrows += [H * 192 + j for j in _DEINT]
                krows += [H * 192 + 64 + j for j in _DEINT]
            qk_rows += qrows + krows
        v_rows = []
        for h in range(4):
            H = heads[h]
            v_rows += [H * 192 + 128 + j for j in range(DK)]
        vch_out = []
        for h in range(4):
            H = heads[h]
            vch_out += [H * 64 + j for j in range(DK)]

        in_maps.append({
            "XT": xt[b],
            "WQK": np.ascontiguousarray(Wqkv[qk_rows].T.astype(ml_dtypes.bfloat16)),
            "WV": np.ascontiguousarray(Wqkv[v_rows].T.astype(ml_dtypes.bfloat16)),
            "WOUT": np.ascontiguousarray(Wout[:, vch_out].T.astype(ml_dtypes.bfloat16)),
            "PSW": psw,
            "CS": cs,
            "TRI": tri,
            "ONES64": np.ones((1, 64), dtype=np.float32),
        })
    return in_maps


_CACHED = {}


def _get_program(debug=False):
    key = bool(debug)
    if key not in _CACHED:
        _CACHED[key] = build_program(debug=debug)
    return _CACHED[key]


def run_cores(inputs, debug=False, trace=False, tmpdir=None):
    nc = _get_program(debug=debug)
    in_maps = shard_inputs(**inputs)
    res = run_bass_kernel_spmd(
        nc, in_maps, core_ids=list(range(NCORES)), trace=trace, tmpdir=tmpdir,
    )
    return res


def combine(results, bout):
    bout = np.asarray(bout, dtype=np.float32)
    out = np.empty((B, T, D), dtype=np.float32)
    for b in range(B):
        acc = results[4 * b]["OUT"].astype(np.float32)
        for c in range(4 * b + 1, 4 * b + 4):
            acc += results[c]["OUT"].astype(np.float32)
        out[b] = acc + bout[None, :]
    return out


def kernel(x, Wqkv, bqkv, Wout, bout):
    res = run_cores(dict(x=x, Wqkv=Wqkv, bqkv=bqkv, Wout=Wout, bout=bout))
    return combine(res.results, bout)
